# revision 5
# baseline (speedup 1.0000x reference)
"""Trainium2 Bass kernel for nn_Metamorph_parameterReinforcer.

Math background (exact identities, verified against the reference):
  The reference's einsum("bfp,mn->bfm", fx, wfft) sums over BOTH p and n,
  so each "STFT block" collapses:
    sum_p fft(x, norm=forward)[..., p] == x[..., 0]
    block(x)[b, f, k] = Re tanh(x[b, f, 0] * W[k]),
       W[k] = sum_m (sum_n wfft[m, n]) * exp(2j*pi*k*m/64)
  Chaining three blocks, only element 0 of the last axis propagates:
    a  = params[:, :, 0]
    s1 = Retanh(a  * W0[0]);  s2 = Retanh(s1 * W1[0])
    x3[b, f, l] = Retanh(s2[b, f] * W2[l])         # (512, 1000, 64)
    h  = tanh(x3.reshape(512, 64000) @ lin1_w.T + lin1_b)
    out = sigmoid(h @ lin2_w.T + lin2_b)
  Because |W0[0]|, |W1[0]| ~ 32000 (sums of 64000 uniforms), tanh saturates
  and s2 is exactly +-1 in f32 for all but (rare) |a| < ~1e-4 entries. Where
  s2 is exactly +-1, x3[b, f, :] = s2[b, f] * X1[:] with X1 = Retanh(W2) --
  exactly rank-1. Rare non-saturated entries are handled by an exact
  correction term dH added before the lin1 tanh (computed on host from the
  few affected (b, f) pairs; zero for typical inputs).

Device kernel (8 cores, lin1_w sharded over its output dim j, 125 rows/core;
the 256 MB lin1_w read is the memory roofline and is read exactly once
across the fleet):
  stage 1: A[j, f] = sum_l X1[l] * w1[j, 64 f + l]      (TensorE)
           K-packs two f per matmul: lhsT = w1 tile [(f', l)=128, j=125],
           rhs = block-diag X1 [(f', l)=128, 2] -> out [j=125, 2] per pair.
  stage 2: A -> A_T via PE transpose; h[j, b] = tanh(sum_f A_T[f, j] *
           s2T[f, b] + lin1_b[j] (+ dH)) -- K=f matmuls + ScalarE tanh.
  stage 3: partial[k, b] = sum_j lin2_w[k, j] * h[j, b]  (one matmul)
Host combines the 8 partials: out = sigmoid(sum_c partial_c + lin2_b).
"""

import numpy as np

B, MODES, L = 512, 1000, 64
NCORES = 8
JSH = MODES // NCORES          # 125 lin1 output rows per core
NG = MODES // 2                # 500 f-pair groups for stage 1
G_CHUNK = 50                   # f-pair groups per DMA chunk (3.125 MB)
SAT = 50.0                     # |2*s*Re(W)| beyond this: Retanh == sign


def _retanh(s, w):
    """Re tanh(s * w) for real array s and complex (array or scalar) w."""
    s = np.asarray(s, np.float64)
    x = 2.0 * np.multiply.outer(s, np.real(w))
    y = 2.0 * np.multiply.outer(s, np.imag(w))
    xc = np.clip(x, -SAT, SAT)
    with np.errstate(over="ignore", invalid="ignore"):
        r = np.sinh(xc) / (np.cosh(xc) + np.cos(y))
    return np.where(np.abs(x) >= SAT, np.sign(x), r)


def _wvec(wre, wim):
    """W[k] = sum_m (sum_n w[m, n]) * exp(2j pi k m / L)."""
    wsum = wre.astype(np.float64).sum(axis=1) + 1j * wim.astype(np.float64).sum(axis=1)
    tw = np.exp(2j * np.pi * np.outer(np.arange(L), np.arange(L)) / L)
    return tw @ wsum


_CACHE = {}


def _build_program(use_dh):
    """Build (and cache) the Bass program. Same program for all 8 cores."""
    key = ("prog", use_dh)
    if key in _CACHE:
        return _CACHE[key]

    import concourse.bacc as bacc
    import concourse.mybir as mybir
    import concourse.tile as tile
    from concourse.masks import make_identity

    f32 = mybir.dt.float32
    nc = bacc.Bacc("TRN2", target_bir_lowering=False, debug=False)

    w1l_d = nc.dram_tensor("w1l", [128, NG, JSH], f32, kind="ExternalInput")
    s2t_d = nc.dram_tensor("s2t", [MODES, B], f32, kind="ExternalInput")
    x1d_d = nc.dram_tensor("x1d", [128, 2], f32, kind="ExternalInput")
    bias_d = nc.dram_tensor("bias", [JSH, 1], f32, kind="ExternalInput")
    l2t_d = nc.dram_tensor("l2t", [JSH, L], f32, kind="ExternalInput")
    if use_dh:
        dht_d = nc.dram_tensor("dht", [JSH, B], f32, kind="ExternalInput")
    outp_d = nc.dram_tensor("outp", [L, B], f32, kind="ExternalOutput")

    n_chunks = (NG + G_CHUNK - 1) // G_CHUNK
    n_ft = (MODES + 127) // 128          # 8 f-tiles for stage 2

    with tile.TileContext(nc) as tc:
        with (
            tc.tile_pool(name="const", bufs=1) as const,
            tc.tile_pool(name="w1pool", bufs=3) as w1pool,
            tc.tile_pool(name="acc", bufs=1) as acc,
            tc.tile_pool(name="psA", bufs=2, space="PSUM") as psA,
            tc.tile_pool(name="psT", bufs=2, space="PSUM") as psT,
            tc.tile_pool(name="psH", bufs=1, space="PSUM") as psH,
            tc.tile_pool(name="psO", bufs=1, space="PSUM") as psO,
        ):
            ident = const.tile([128, 128], f32)
            make_identity(nc, ident[:])

            x1d = const.tile([128, 2], f32)
            nc.sync.dma_start(x1d[:], x1d_d.ap())
            bias = const.tile([JSH, 1], f32)
            nc.sync.dma_start(bias[:], bias_d.ap())
            l2t = const.tile([JSH, L], f32)
            nc.sync.dma_start(l2t[:], l2t_d.ap())
            s2t = const.tile([128, n_ft * B], f32)
            for t in range(n_ft):
                ft = min(128, MODES - 128 * t)
                nc.sync.dma_start(
                    s2t[0:ft, B * t : B * (t + 1)],
                    s2t_d.ap()[128 * t : 128 * t + ft, :],
                )
            if use_dh:
                dht = const.tile([JSH, B], f32)
                nc.sync.dma_start(dht[:], dht_d.ap())

            # ---- stage 1: A[j, f] = sum_l X1[l] w1[j, 64 f + l] ----
            a_sb = acc.tile([JSH, MODES], f32)
            # psum tile 0 covers g in [0, 256), tile 1 covers [256, 500)
            pa = [
                psA.tile([JSH, 512], f32, name=f"pa{i}", tag=f"pa{i}")
                for i in range(2)
            ]
            for c in range(n_chunks):
                g0 = c * G_CHUNK
                gn = min(G_CHUNK, NG - g0)
                w1c = w1pool.tile([128, G_CHUNK, JSH], f32, tag="w1c")
                nc.sync.dma_start(
                    w1c[:, 0:gn, :], w1l_d.ap()[:, g0 : g0 + gn, :]
                )
                for gg in range(gn):
                    g = g0 + gg
                    half = g // 256
                    col = 2 * (g - half * 256)
                    nc.tensor.matmul(
                        pa[half][:, col : col + 2],
                        w1c[:, gg, :],
                        x1d[:],
                        start=True,
                        stop=True,
                    )
            nc.vector.tensor_copy(a_sb[:, 0:512], pa[0][:, :])
            nc.vector.tensor_copy(a_sb[:, 512:MODES], pa[1][:, 0 : MODES - 512])

            # ---- transpose A -> A_T[f, j] ----
            at_sb = acc.tile([128, n_ft * JSH], f32)
            for t in range(n_ft):
                ft = min(128, MODES - 128 * t)
                pt = psT.tile([128, JSH], f32, tag="pt")
                nc.tensor.transpose(
                    pt[0:ft, :],
                    a_sb[:, 128 * t : 128 * t + ft],
                    ident[0:JSH, 0:JSH],
                )
                nc.vector.tensor_copy(
                    at_sb[0:ft, JSH * t : JSH * (t + 1)], pt[0:ft, :]
                )

            # ---- stage 2: h[j, b] = tanh(sum_f A_T[f, j] s2t[f, b] + bias) ----
            ph = psH.tile([JSH, B], f32)
            for t in range(n_ft):
                ft = min(128, MODES - 128 * t)
                nc.tensor.matmul(
                    ph[:, :],
                    at_sb[0:ft, JSH * t : JSH * (t + 1)],
                    s2t[0:ft, B * t : B * (t + 1)],
                    start=(t == 0),
                    stop=(t == n_ft - 1),
                )
            if use_dh:
                nc.vector.tensor_add(ph[:, :], ph[:, :], dht[:, :])
            h_sb = acc.tile([JSH, B], f32)
            nc.scalar.activation(
                h_sb[:, :],
                ph[:, :],
                mybir.ActivationFunctionType.Tanh,
                bias=bias[:, 0:1],
            )

            # ---- stage 3: partial[k, b] = sum_j l2t[j, k] h[j, b] ----
            po = psO.tile([L, B], f32)
            nc.tensor.matmul(po[:, :], l2t[:, :], h_sb[:, :], start=True, stop=True)
            o_sb = acc.tile([L, B], f32)
            nc.vector.tensor_copy(o_sb[:, :], po[:, :])
            nc.sync.dma_start(outp_d.ap(), o_sb[:, :])

    nc.compile()
    _CACHE[key] = nc
    return nc


def profile_last(trace_cores=None):
    """Re-run the last-built program with NTFF tracing (dev/test helper)."""
    if "last_run" not in _CACHE:
        return None
    from concourse.bass_utils import run_bass_kernel_spmd

    nc, in_maps = _CACHE["last_run"]
    return run_bass_kernel_spmd(
        nc,
        in_maps,
        list(range(NCORES)),
        trace=True,
        trace_cores=trace_cores,
    )


def kernel(
    params,
    wfft0_re,
    wfft0_im,
    wfft1_re,
    wfft1_im,
    wfft2_re,
    wfft2_im,
    lin1_w,
    lin1_b,
    lin2_w,
    lin2_b,
):
    from concourse.bass_utils import run_bass_kernel_spmd

    # ---- host: closed-form collapse of the three spectral blocks ----
    a = params[:, :, 0].astype(np.float64)
    w0 = _wvec(wfft0_re, wfft0_im)[0]
    w1v = _wvec(wfft1_re, wfft1_im)[0]
    w2 = _wvec(wfft2_re, wfft2_im)
    s1 = _retanh(a, w0)
    s2 = _retanh(s1, w1v).astype(np.float32)
    x1 = _retanh(np.float64(1.0), w2).astype(np.float32)  # (64,)

    # exact correction for entries where tanh did not saturate to +-1
    bad_b, bad_f = np.nonzero(np.abs(s2) != np.float32(1.0))
    use_dh = bad_b.size > 0
    dh = None
    if use_dh:
        dh = np.zeros((B, MODES), np.float64)
        x1_64 = x1.astype(np.float64)
        for b, f in zip(bad_b.tolist(), bad_f.tolist()):
            s = np.float64(s2[b, f])
            delta = _retanh(s, w2)[0] - s * x1_64
            dh[b, :] += lin1_w[:, 64 * f : 64 * (f + 1)].astype(np.float64) @ delta
        dh = dh.astype(np.float32)

    # ---- host: per-core shards / layouts ----
    s2t = np.ascontiguousarray(s2.T)                      # (1000, 512)
    x1d = np.zeros((128, 2), np.float32)
    x1d[0:64, 0] = x1
    x1d[64:128, 1] = x1

    in_maps = []
    for c in range(NCORES):
        j0, j1 = JSH * c, JSH * (c + 1)
        shard = lin1_w[j0:j1]                             # (125, 64000)
        w1l = np.ascontiguousarray(
            shard.reshape(JSH, NG, 2, L).transpose(2, 3, 1, 0).reshape(128, NG, JSH)
        )
        m = {
            "w1l": w1l,
            "s2t": s2t,
            "x1d": x1d,
            "bias": np.ascontiguousarray(lin1_b[j0:j1].reshape(JSH, 1)),
            "l2t": np.ascontiguousarray(lin2_w[:, j0:j1].T),
        }
        if use_dh:
            m["dht"] = np.ascontiguousarray(dh[:, j0:j1].T)
        in_maps.append(m)

    nc = _build_program(use_dh)
    _CACHE["last_run"] = (nc, in_maps)
    res = run_bass_kernel_spmd(nc, in_maps, list(range(NCORES)))

    acc = np.zeros((L, B), np.float64)
    for c in range(NCORES):
        acc += res.results[c]["outp"].astype(np.float64)
    out = 1.0 / (1.0 + np.exp(-(acc.T + lin2_b.astype(np.float64))))
    return out.astype(np.float32)


# revision 26
# speedup vs baseline: 2.7938x; 2.7938x over previous
"""Trainium2 Bass kernel for nn_Metamorph_parameterReinforcer.

Math background (exact identities, verified against the reference):
  The reference's einsum("bfp,mn->bfm", fx, wfft) sums over BOTH p and n,
  so each "STFT block" collapses:
    sum_p fft(x, norm=forward)[..., p] == x[..., 0]
    block(x)[b, f, k] = Re tanh(x[b, f, 0] * W[k]),
       W[k] = sum_m (sum_n wfft[m, n]) * exp(2j*pi*k*m/64)
  Chaining three blocks, only element 0 of the last axis propagates:
    a  = params[:, :, 0]
    s1 = Retanh(a  * W0[0]);  s2 = Retanh(s1 * W1[0])
    x3[b, f, l] = Retanh(s2[b, f] * W2[l])         # (512, 1000, 64)
    h  = tanh(x3.reshape(512, 64000) @ lin1_w.T + lin1_b)
    out = sigmoid(h @ lin2_w.T + lin2_b)
  Because |W0[0]|, |W1[0]| ~ 32000 (sums of 64000 uniforms), tanh saturates
  and s2 is exactly +-1 in f32 for all but (rare) |a| < ~1e-4 entries. Where
  s2 is exactly +-1, x3[b, f, :] = s2[b, f] * X1[:] with X1 = Retanh(W2) --
  exactly rank-1. Rare non-saturated entries are handled by an exact
  correction term dH added before the lin1 tanh (computed on host from the
  few affected (b, f) pairs; zero for typical inputs).

Device kernel (8 cores, lin1_w sharded over its output dim j, 125 rows/core;
the 256 MB lin1_w read is the memory roofline and is read exactly once
across the fleet):
  stage 1: A[j, f] = sum_l X1[l] * w1[j, 64 f + l]      (TensorE)
           K-packs two f per matmul: lhsT = w1 tile [(f', l)=128, j=125],
           rhs = block-diag X1 [(f', l)=128, 2] -> out [j=125, 2] per pair.
  stage 2: A -> A_T via PE transpose; h[j, b] = tanh(sum_f A_T[f, j] *
           s2T[f, b] + lin1_b[j] (+ dH)) -- K=f matmuls + ScalarE tanh.
  stage 3: partial[k, b] = sum_j lin2_w[k, j] * h[j, b]  (one matmul)
Host combines the 8 partials: out = sigmoid(sum_c partial_c + lin2_b).
"""

import numpy as np

B, MODES, L = 512, 1000, 64
NCORES = 8
JSH = MODES // NCORES          # 125 lin1 output rows per core
NGRP = MODES // 4              # 250 four-f groups for the M4 stage 1
NTOT = NGRP * JSH              # 31250 stage-1 outputs (g, j) per core
NCH = 4 * JSH                  # psum chunk: 4 g x 125 j = 500 columns
BIGCH = 5 * NCH                # DMA chunk (2500 cols x 2 halves, 1.25 MB)
SAT = 50.0                     # |2*s*Re(W)| beyond this: Retanh == sign
SAT = 50.0                     # |2*s*Re(W)| beyond this: Retanh == sign


def _retanh(s, w):
    """Re tanh(s * w) for real array s and complex (array or scalar) w."""
    s = np.asarray(s, np.float64)
    x = 2.0 * np.multiply.outer(s, np.real(w))
    y = 2.0 * np.multiply.outer(s, np.imag(w))
    xc = np.clip(x, -SAT, SAT)
    with np.errstate(over="ignore", invalid="ignore"):
        r = np.sinh(xc) / (np.cosh(xc) + np.cos(y))
    return np.where(np.abs(x) >= SAT, np.sign(x), r)


def _wvec(wre, wim):
    """W[k] = sum_m (sum_n w[m, n]) * exp(2j pi k m / L)."""
    wsum = wre.astype(np.float64).sum(axis=1) + 1j * wim.astype(np.float64).sum(axis=1)
    tw = np.exp(2j * np.pi * np.outer(np.arange(L), np.arange(L)) / L)
    return tw @ wsum


_CACHE = {}


def _build_program(use_dh):
    """Build (and cache) the Bass program. Same program for all 8 cores."""
    key = ("prog", use_dh, "m4v3", NCH, BIGCH)
    if key in _CACHE:
        return _CACHE[key]

    import concourse.bacc as bacc
    import concourse.mybir as mybir
    import concourse.tile as tile

    f32 = mybir.dt.float32
    bf16 = mybir.dt.bfloat16
    nc = bacc.Bacc("TRN2", target_bir_lowering=False, debug=False)

    w1x_d = nc.dram_tensor("w1x", [128, 2, NTOT], bf16, kind="ExternalInput")
    s2t_d = nc.dram_tensor("s2t", [MODES, B], bf16, kind="ExternalInput")
    x1d4_d = nc.dram_tensor("x1d4", [128, 8], bf16, kind="ExternalInput")
    bias_d = nc.dram_tensor("bias", [JSH, 1], f32, kind="ExternalInput")
    l2t_d = nc.dram_tensor("l2t", [JSH, L], f32, kind="ExternalInput")
    if use_dh:
        dht_d = nc.dram_tensor("dht", [JSH, B], f32, kind="ExternalInput")
    outp_d = nc.dram_tensor("outp", [L, B], f32, kind="ExternalOutput")

    n_ft = (MODES + 127) // 128          # 8 f-tiles for stage 2

    with tile.TileContext(nc) as tc:
        with (
            tc.tile_pool(name="const", bufs=1) as const,
            tc.tile_pool(name="w1pool", bufs=5) as w1pool,
            tc.tile_pool(name="acc", bufs=1) as acc,
            tc.tile_pool(name="psC", bufs=3, space="PSUM") as psC,
            tc.tile_pool(name="psH", bufs=1, space="PSUM") as psH,
            tc.tile_pool(name="psO", bufs=1, space="PSUM") as psO,
        ):
            x1d4 = const.tile([128, 8], bf16)
            nc.sync.dma_start(x1d4[:], x1d4_d.ap())
            bias = const.tile([JSH, 1], f32)
            nc.sync.dma_start(bias[:], bias_d.ap())
            l2t = const.tile([JSH, L], f32)
            nc.sync.dma_start(l2t[:], l2t_d.ap())
            s2t = const.tile([128, n_ft * B], bf16)
            for t in range(n_ft):
                ft = min(128, MODES - 128 * t)
                nc.scalar.dma_start(
                    s2t[0:ft, B * t : B * (t + 1)],
                    s2t_d.ap()[128 * t : 128 * t + ft, :],
                )
            if use_dh:
                dht = const.tile([JSH, B], f32)
                nc.sync.dma_start(dht[:], dht_d.ap())

            # ---- stage 1 (TensorE): S[fp, g, j] = sum_l X1[l] w1[j, 4g+fp, l]
            # lhsT = block-diag X1 halves [K=(fp,lh)=128, 4]; rhs = w1x
            # chunks [128, 500]; two matmuls (l low/high) accumulate in PSUM.
            s4 = acc.tile([4, NGRP, JSH], bf16)
            dma_engines = [nc.sync, nc.scalar]
            ev = 0
            n_big = (NTOT + BIGCH - 1) // BIGCH
            for bc in range(n_big):
                n0 = bc * BIGCH
                nn_big = min(BIGCH, NTOT - n0)
                w1c = w1pool.tile([128, 2, BIGCH], bf16, tag="w1c")
                dma_engines[bc % 2].dma_start(
                    w1c[:, :, 0:nn_big], w1x_d.ap()[:, :, n0 : n0 + nn_big]
                )
                for off in range(0, nn_big, NCH):
                    nn = min(NCH, nn_big - off)
                    gn = nn // JSH
                    g0 = (n0 + off) // JSH
                    pc = psC.tile([4, NCH], f32, tag="pc")
                    nc.tensor.matmul(
                        pc[0:4, 0:nn],
                        x1d4[:, 0:4],
                        w1c[:, 0, off : off + nn],
                        start=True,
                        stop=False,
                    )
                    nc.tensor.matmul(
                        pc[0:4, 0:nn],
                        x1d4[:, 4:8],
                        w1c[:, 1, off : off + nn],
                        start=False,
                        stop=True,
                    )
                    src = pc[0:4, 0:nn].rearrange("p (g j) -> p g j", j=JSH)
                    dst = s4[0:4, g0 : g0 + gn, :]
                    if ev % 2 == 0:
                        nc.vector.tensor_copy(dst, src)
                    else:
                        nc.scalar.activation(
                            dst, src, mybir.ActivationFunctionType.Copy
                        )
                    ev += 1

            # ---- scatter S[fp, g, j] -> A_T[fhat = 250 fp + g, j] ----
            # (stage 2 contracts over fhat; s2t rows are host-permuted to match)
            at_sb = acc.tile([128, n_ft * JSH], bf16)
            for fp in range(4):
                a = 250 * fp
                end = 250 * (fp + 1)
                while a < end:
                    t = a // 128
                    b_ = min(end, 128 * (t + 1))
                    p0 = a - 128 * t
                    ln = b_ - a
                    g0 = a - 250 * fp
                    nc.gpsimd.dma_start(
                        at_sb[p0 : p0 + ln, JSH * t : JSH * (t + 1)],
                        s4[fp : fp + 1, g0 : g0 + ln, :],
                    )
                    a = b_

            # ---- stage 2: h[j, b] = tanh(sum_f A_T[f, j] s2t[f, b] + bias) ----
            ph = psH.tile([JSH, B], f32)
            for t in range(n_ft):
                ft = min(128, MODES - 128 * t)
                nc.tensor.matmul(
                    ph[:, :],
                    at_sb[0:ft, JSH * t : JSH * (t + 1)],
                    s2t[0:ft, B * t : B * (t + 1)],
                    start=(t == 0),
                    stop=(t == n_ft - 1),
                )
            if use_dh:
                nc.vector.tensor_add(ph[:, :], ph[:, :], dht[:, :])
            h_sb = acc.tile([JSH, B], f32)
            nc.scalar.activation(
                h_sb[:, :],
                ph[:, :],
                mybir.ActivationFunctionType.Tanh,
                bias=bias[:, 0:1],
            )

            # ---- stage 3: partial[k, b] = sum_j l2t[j, k] h[j, b] ----
            po = psO.tile([L, B], f32)
            nc.tensor.matmul(po[:, :], l2t[:, :], h_sb[:, :], start=True, stop=True)
            o_sb = acc.tile([L, B], f32)
            nc.vector.tensor_copy(o_sb[:, :], po[:, :])
            nc.sync.dma_start(outp_d.ap(), o_sb[:, :])

    nc.compile()
    _CACHE[key] = nc
    return nc


def profile_last(trace_cores=None):
    """Re-run the last-built program with NTFF tracing (dev/test helper)."""
    if "last_run" not in _CACHE:
        return None
    from concourse.bass_utils import run_bass_kernel_spmd

    nc, in_maps = _CACHE["last_run"]
    return run_bass_kernel_spmd(
        nc,
        in_maps,
        list(range(NCORES)),
        trace=True,
        trace_cores=trace_cores,
    )


def kernel(
    params,
    wfft0_re,
    wfft0_im,
    wfft1_re,
    wfft1_im,
    wfft2_re,
    wfft2_im,
    lin1_w,
    lin1_b,
    lin2_w,
    lin2_b,
):
    from concourse.bass_utils import run_bass_kernel_spmd

    # ---- host: closed-form collapse of the three spectral blocks ----
    a = params[:, :, 0].astype(np.float64)
    w0 = _wvec(wfft0_re, wfft0_im)[0]
    w1v = _wvec(wfft1_re, wfft1_im)[0]
    w2 = _wvec(wfft2_re, wfft2_im)
    s1 = _retanh(a, w0)
    s2 = _retanh(s1, w1v).astype(np.float32)
    x1 = _retanh(np.float64(1.0), w2).astype(np.float32)  # (64,)

    # exact correction for entries where tanh did not saturate to +-1
    bad_b, bad_f = np.nonzero(np.abs(s2) != np.float32(1.0))
    use_dh = bad_b.size > 0
    dh = None
    if use_dh:
        dh = np.zeros((B, MODES), np.float64)
        x1_64 = x1.astype(np.float64)
        for b, f in zip(bad_b.tolist(), bad_f.tolist()):
            s = np.float64(s2[b, f])
            delta = _retanh(s, w2)[0] - s * x1_64
            dh[b, :] += lin1_w[:, 64 * f : 64 * (f + 1)].astype(np.float64) @ delta
        dh = dh.astype(np.float32)

    # ---- host: per-core shards / layouts ----
    import ml_dtypes

    bf16 = ml_dtypes.bfloat16
    # stage-2 contraction order fhat = 250*fp + g  <->  f = 4*g + fp
    fhat = np.arange(MODES)
    perm = 4 * (fhat % 250) + fhat // 250
    s2t = np.ascontiguousarray(s2.T[perm].astype(bf16))   # (1000, 512)

    x1d4 = np.zeros((128, 8), np.float32)
    for h in range(2):
        for fp in range(4):
            x1d4[fp * 32 : (fp + 1) * 32, 4 * h + fp] = x1[32 * h : 32 * (h + 1)]
    x1d4 = x1d4.astype(bf16)

    in_maps = []
    for c in range(NCORES):
        j0, j1 = JSH * c, JSH * (c + 1)
        # w1x[half, (fp, lh), (g, j)] = w1[j, 64*(4g+fp) + 32*half + lh]
        # [K=(fp,lh)=128, half, N=(g,j)]
        w1x = np.ascontiguousarray(
            lin1_w[j0:j1]
            .reshape(JSH, NGRP, 4, 2, 32)
            .transpose(2, 4, 3, 1, 0)
            .reshape(128, 2, NTOT)
            .astype(bf16)
        )
        m = {
            "w1x": w1x,
            "s2t": s2t,
            "x1d4": x1d4,
            "bias": np.ascontiguousarray(lin1_b[j0:j1].reshape(JSH, 1)),
            "l2t": np.ascontiguousarray(lin2_w[:, j0:j1].T),
        }
        if use_dh:
            m["dht"] = np.ascontiguousarray(dh[:, j0:j1].T)
        in_maps.append(m)

    nc = _build_program(use_dh)
    _CACHE["last_run"] = (nc, in_maps)
    res = run_bass_kernel_spmd(nc, in_maps, list(range(NCORES)))

    acc = np.zeros((L, B), np.float64)
    for c in range(NCORES):
        acc += res.results[c]["outp"].astype(np.float64)
    out = 1.0 / (1.0 + np.exp(-(acc.T + lin2_b.astype(np.float64))))
    return out.astype(np.float32)
